# revision 20
# baseline (speedup 1.0000x reference)
"""Trainium2 Bass kernel for nn_Channel_Attention (XCA-style windowed channel attention).

v3 = v2 + software pipelining: the attention phase of strip s-1 is emitted
interleaved with the qkv/depthwise GEMM stages of strip s (disjoint PSUM
banks, double-buffered qk/v tiles), so the PE never idles long enough for
HAM to re-throttle and DVE/ACT chains hide under PE work.

Kernel math (per core, data-parallel over batch x H-half):
  - stage A: qkv1 = conv1x1(x) on a 10-row halo strip (bf16 GEMM, K=192)
  - stage B: qkv_dw = sum_t diag(dw[:,t]) @ shift_t(qkv1)  (9 accumulating bf16
    matmuls per chunk, K = chunk size <= 128)
  - per-(chan,window) l2 norms: ACT Square + DVE window reduce + Ln + Exp(-0.5 ln)
  - q,k evacuated f32 with fused 1/norm scaling (DVE stt, broadcast AP)
  - per-window pc layout via PE transpose (f32: bf16 PSUM is broken on trn2 HW)
  - transposed gram G2[j,i] with k zero-padded to K=128 (bf16 accum groups with
    mixed tile positions crash trn2; all matmuls full-K at position (0,0))
  - exp feeds out-matmul as lhsT directly; ones column in v gives Z in-matmul
  - 1/Z fused into evacuation; final proj GEMM restores flat pixel order
"""
import sys

sys.path.insert(0, "/opt/trn_rl_repo")

import numpy as np
import ml_dtypes

BF16 = ml_dtypes.bfloat16

DIM, HEADS, PS = 192, 6, 8
B, H, W = 4, 256, 256
C3 = 3 * DIM  # 576
NCORES = 8
ROWS = 128
NSTRIP = 16

_CACHE = {}
LAST_RESULT = None


def _build_nc():
    import contextlib

    import concourse.mybir as mybir
    import concourse.tile as tile
    from concourse import bacc

    f32 = mybir.dt.float32
    bf = mybir.dt.bfloat16
    AL = mybir.AluOpType
    AF = mybir.ActivationFunctionType
    AX = mybir.AxisListType

    nc = bacc.Bacc("TRN2", target_bir_lowering=False, debug=False, num_devices=NCORES)

    xp_e = nc.declare_dram_parameter("xp", [DIM, 130, 258], bf, isOutput=False)
    wqA_e = nc.declare_dram_parameter("wqA", [128, C3], bf, isOutput=False)
    wqB_e = nc.declare_dram_parameter("wqB", [64, C3], bf, isOutput=False)
    dgA_e = nc.declare_dram_parameter("dgA", [128, 36 * 128], bf, isOutput=False)
    dgB_e = nc.declare_dram_parameter("dgB", [64, 9 * 64], bf, isOutput=False)
    pjA_e = nc.declare_dram_parameter("pjA", [128, 192], bf, isOutput=False)
    pjB_e = nc.declare_dram_parameter("pjB", [64, 192], bf, isOutput=False)
    tmpA_e = nc.declare_dram_parameter("tmpA", [128, 1], f32, isOutput=False)
    tmpB_e = nc.declare_dram_parameter("tmpB", [64, 1], f32, isOutput=False)
    idn_e = nc.declare_dram_parameter("idn", [128, 128], bf, isOutput=False)
    mskA_e = nc.declare_dram_parameter("mskA", [128, 512], bf, isOutput=False)
    mskB_e = nc.declare_dram_parameter("mskB", [64, 256], bf, isOutput=False)
    y_e = nc.declare_dram_parameter("y", [DIM, ROWS, 256], bf, isOutput=True)

    with tile.TileContext(nc) as tc, contextlib.ExitStack() as ctx:
        const = ctx.enter_context(tc.tile_pool(name="const", bufs=1))
        xs_p = ctx.enter_context(tc.tile_pool(name="xs", bufs=2))
        q1_p = ctx.enter_context(tc.tile_pool(name="q1", bufs=1))
        qk_p = ctx.enter_context(tc.tile_pool(name="qk", bufs=2))
        pc_p = ctx.enter_context(tc.tile_pool(name="pc", bufs=1))
        sm_p = ctx.enter_context(tc.tile_pool(name="small", bufs=2))
        stg_p = ctx.enter_context(tc.tile_pool(name="stg", bufs=2))
        e_p = ctx.enter_context(tc.tile_pool(name="eb", bufs=2))
        att_p = ctx.enter_context(tc.tile_pool(name="att", bufs=1))
        y_p = ctx.enter_context(tc.tile_pool(name="yb", bufs=1))
        ps_big = ctx.enter_context(tc.tile_pool(name="psbig", bufs=4, space="PSUM"))
        ps_pct = ctx.enter_context(tc.tile_pool(name="pspct", bufs=1, space="PSUM"))
        ps_gA = ctx.enter_context(tc.tile_pool(name="psga", bufs=1, space="PSUM"))
        ps_gB = ctx.enter_context(tc.tile_pool(name="psgb", bufs=1, space="PSUM"))
        ps_oA = ctx.enter_context(tc.tile_pool(name="psoa", bufs=1, space="PSUM"))

        # ---- constants ----
        wqA = const.tile([128, C3], bf)
        nc.sync.dma_start(wqA[:], wqA_e[:, :])
        wqB = const.tile([64, C3], bf)
        nc.sync.dma_start(wqB[:], wqB_e[:, :])
        dgA = const.tile([128, 36 * 128], bf)
        nc.sync.dma_start(dgA[:], dgA_e[:, :])
        dgB = const.tile([64, 9 * 64], bf)
        nc.sync.dma_start(dgB[:], dgB_e[:, :])
        pjA = const.tile([128, 192], bf)
        nc.sync.dma_start(pjA[:], pjA_e[:, :])
        pjB = const.tile([64, 192], bf)
        nc.sync.dma_start(pjB[:], pjB_e[:, :])
        tmpA = const.tile([128, 1], f32)
        nc.sync.dma_start(tmpA[:], tmpA_e[:, :])
        tmpB = const.tile([64, 1], f32)
        nc.sync.dma_start(tmpB[:], tmpB_e[:, :])
        idn = const.tile([128, 128], bf)
        nc.sync.dma_start(idn[:], idn_e[:, :])
        mskA = const.tile([128, 512], bf)
        nc.sync.dma_start(mskA[:], mskA_e[:, :])
        mskB = const.tile([64, 256], bf)
        nc.sync.dma_start(mskB[:], mskB_e[:, :])

        MCH = [(0, 128), (128, 128), (256, 128), (384, 128), (512, 64)]

        # k in pc layout, zero-padded to full 128 partitions per window parity
        kzE = pc_p.tile([128, 8 * 192], bf, name="kzE")
        kzO = pc_p.tile([128, 8 * 192], bf, name="kzO")
        nc.vector.memset(kzE[64:128, :], 0.0)
        nc.vector.memset(kzO[0:64, :], 0.0)

        # qkv1 (pre-depthwise) chunk tiles: [chans, 10 rows x 258 cols]
        q1 = [q1_p.tile([128, 2580], bf, name=f"q1c{m}") for m in range(4)]
        q1.append(q1_p.tile([64, 2580], bf, name="q1c4"))
        q1v = [t[0:msz, :].rearrange("p (r c) -> p r c", r=10)
               for t, (mo, msz) in zip(q1, MCH)]

        state = {}  # per-strip tiles handed from gen_AB(s) to gen_att(s)

        def gen_AB(s):
            # ---- load x strip (10 rows x 258 cols, padded) ----
            xsA = xs_p.tile([128, 10 * 258], bf, tag="xsA")
            nc.sync.dma_start(xsA[:], xp_e[0:128, 8 * s:8 * s + 10, :])
            xsB = xs_p.tile([64, 10 * 258], bf, tag="xsB")
            nc.sync.dma_start(xsB[:], xp_e[128:192, 8 * s:8 * s + 10, :])

            qk = [qk_p.tile([128, 2048], bf, name=f"qk{m}", tag=f"qk{m}")
                  for m in range(3)]
            # v tiles: 65-col window stride, col 64 of each window = 1.0 so
            # out-matmuls against [v | 1] produce Z in the extra column.
            v3 = qk_p.tile([128, 32 * 65], bf, tag="v3")
            v4 = qk_p.tile([64, 32 * 65], bf, tag="v4")
            v3w = v3[:].rearrange("p (w c) -> p w c", w=32)
            v4w = v4[0:64, :].rearrange("p (w c) -> p w c", w=32)
            nc.vector.memset(v3w[:, :, 64:65], 1.0)
            nc.vector.memset(v4w[:, :, 64:65], 1.0)
            state[s] = (qk, v3, v4)

            # ---- stage A: qkv1 = conv1x1(x) ----
            # Strip 0 computes the full 10-row halo strip; later strips reuse
            # the previous strip's last 2 rows (q1 rows 8,9 == this strip's
            # rows 0,1) via an SBUF copy and compute only the 8 new rows.
            if s == 0:
                blocks = [(i * 430, 430) for i in range(6)]
            else:
                for m, (mo, msz) in enumerate(MCH):
                    nc.vector.tensor_copy(q1[m][0:msz, 0:516],
                                          q1[m][0:msz, 2064:2580])
                blocks = [(516 + i * 344, 344) for i in range(6)]
            for m, (mo, msz) in enumerate(MCH):
                for ng in range(3):
                    psq = [ps_big.tile([128, 512], f32, tag="big",
                                       name=f"psq{m}_{ng}_{i}")
                           for i in range(2)]
                    for kc in range(2):
                        wq = (wqA, wqB)[kc]
                        xf = (xsA, xsB)[kc]
                        for nb, ps in enumerate(psq):
                            c0, wdt = blocks[2 * ng + nb]
                            nc.tensor.matmul(
                                ps[0:msz, 0:wdt], wq[:, mo:mo + msz],
                                xf[:, c0:c0 + wdt],
                                start=(kc == 0), stop=(kc == 1))
                    for nb, ps in enumerate(psq):
                        c0, wdt = blocks[2 * ng + nb]
                        if nb == 0 and ng != 1:
                            nc.vector.tensor_copy(q1[m][0:msz, c0:c0 + wdt],
                                                  ps[0:msz, 0:wdt])
                        else:
                            nc.scalar.copy(q1[m][0:msz, c0:c0 + wdt],
                                           ps[0:msz, 0:wdt])
                    yield

            # ---- stage B: depthwise via diag matmuls + norms + evac ----
            # N-block j = 8 rows x cols [64j, 64j+64) = windows [8j, 8j+8)
            for m, (mo, msz) in enumerate(MCH):
                dg, dbase = (dgA, 9 * m) if m < 4 else (dgB, 0)
                psl = []
                for j in range(4):
                    ps = ps_big.tile([128, 512], f32, tag="big",
                                     name=f"psb{m}_{j}")
                    psl.append((j, ps))
                for t in range(9):
                    dy, dx = t // 3, t % 3
                    lhsT = dg[:, (dbase + t) * msz:(dbase + t + 1) * msz]
                    for j, ps in psl:
                        rhs = q1v[m][:, dy:dy + 8,
                                     64 * j + dx:64 * j + dx + 64
                                     ].rearrange("p r (w c) -> p w r c", w=8)
                        nc.tensor.matmul(
                            ps[0:msz, :], lhsT, rhs,
                            start=(t == 0), stop=(t == 8))
                if m < 3:
                    # fast staging copy frees the PSUM banks immediately;
                    # norms + scaled evac then read SBUF
                    stg = stg_p.tile([128, 2048], bf, tag="stg")
                    for j, ps in psl:
                        if j < 2:
                            nc.vector.tensor_copy(
                                stg[0:msz, 512 * j:512 * (j + 1)], ps[0:msz, :])
                        else:
                            nc.scalar.copy(
                                stg[0:msz, 512 * j:512 * (j + 1)], ps[0:msz, :])
                    yield
                    n2 = sm_p.tile([128, 32], f32, tag="n2")
                    for j, ps in psl:
                        sq = sm_p.tile([128, 512], f32, tag="sq")
                        nc.scalar.activation(sq[0:msz, :],
                                             stg[0:msz, 512 * j:512 * (j + 1)],
                                             AF.Square)
                        sqv = sq[0:msz, :].rearrange("p (w c) -> p w c", w=8)
                        nc.vector.tensor_reduce(
                            n2[0:msz, 8 * j:8 * j + 8], sqv,
                            axis=AX.X, op=AL.add)
                    yield
                    lnn = sm_p.tile([128, 32], f32, tag="nrm")
                    nc.scalar.activation(lnn[0:msz, :], n2[0:msz, :],
                                         AF.Ln)
                    rj = sm_p.tile([128, 32], f32, tag="rj")
                    nc.scalar.activation(rj[0:msz, :], lnn[0:msz, :],
                                         AF.Exp, scale=-0.5)
                    for j, ps in psl:
                        # evac with 1/norm scaling (win-grouped layout)
                        dst = qk[m][:, 512 * j:512 * (j + 1)].rearrange(
                            "p (w c) -> p w c", w=8)
                        nc.vector.scalar_tensor_tensor(
                            dst,
                            stg[0:msz, 512 * j:512 * (j + 1)].rearrange(
                                "p (w c) -> p w c", w=8),
                            1.0,
                            rj[0:msz, 8 * j:8 * j + 8].unsqueeze(2)
                            .broadcast_to((msz, 8, 64)),
                            AL.mult, AL.mult)
                    yield
                else:
                    for j, ps in psl:
                        vw = (v3w, v4w)[m - 3]
                        nc.scalar.copy(
                            vw[0:msz, 8 * j:8 * j + 8, 0:64],
                            ps[0:msz, :].rearrange("p (w c) -> p w c", w=8))
                    yield

        def gen_att(s):
            qk, v3, v4 = state.pop(s)
            # ---- pc transposes + attention, halves of 16 windows ----
            for half in range(2):
                if half == 0:
                    att_A = att_p.tile([128, 2048], bf, tag="attA")
                    att_B = att_p.tile([64, 2048], bf, tag="attB")
                qkpc = pc_p.tile([128, 8 * 192], bf, tag="qkpc")
                for p in range(8):
                    w0 = 16 * half + 2 * p
                    pcps = ps_pct.tile([128, 384], f32, tag="pct")
                    for (srct, oc) in ((qk[0], 0), (qk[1], 128), (qk[2], 256)):
                        inap = srct[0:128, 64 * w0:64 * w0 + 128]
                        # transpose as a regular bf16 matmul: in^T @ I
                        nc.tensor.matmul(pcps[:, oc:oc + 128], inap, idn[:, :],
                                         start=True, stop=True,
                                         skip_group_check=True)
                    nc.vector.tensor_copy(qkpc[:, 192 * p:192 * (p + 1)],
                                          pcps[:, 0:192])
                    nc.vector.tensor_copy(kzE[0:64, 192 * p:192 * (p + 1)],
                                          pcps[0:64, 192:384])
                    nc.scalar.copy(kzO[64:128, 192 * p:192 * (p + 1)],
                                   pcps[64:128, 192:384])
                    if p % 2 == 1:
                        yield
                # grams (transposed: G2[j,i]) + attention, quarters of 4 windows
                for quad in range(4):
                    gA = ps_gA.tile([128, 512], f32, tag="ga")
                    gB = ps_gB.tile([64, 512], f32, tag="gb")
                    for wq in range(4):
                        wl = 4 * quad + wq
                        p, wsub = wl // 2, wl % 2
                        kz = (kzE, kzO)[wsub]
                        qa = qkpc[0:128, 192 * p:192 * p + 128]
                        ka = kz[0:128, 192 * p:192 * p + 128]
                        nc.tensor.matmul(gA[:, 128 * wq:128 * wq + 128], ka, qa,
                                         start=(wq == 0), stop=(wq == 3),
                                         skip_group_check=True)
                        qb = qkpc[0:128, 192 * p + 128:192 * p + 192]
                        kb = kz[0:128, 192 * p + 128:192 * p + 192]
                        nc.tensor.matmul(gB[0:64, 64 * wq:64 * wq + 64], kb, qb,
                                         start=(wq == 0), stop=(wq == 3),
                                         skip_group_check=True)
                    eA = e_p.tile([128, 512], bf, tag="eA")
                    eB = e_p.tile([64, 256], bf, tag="eB")
                    nc.scalar.activation(eA[:], gA[:], AF.Exp, scale=tmpA[:])
                    nc.scalar.activation(eB[0:64, :], gB[0:64, 0:256], AF.Exp,
                                         scale=tmpB[:])
                    eAm = e_p.tile([128, 512], bf, tag="eAm")
                    eBm = e_p.tile([64, 256], bf, tag="eBm")
                    nc.vector.tensor_tensor(eAm[:], eA[:], mskA[:], op=AL.mult)
                    nc.vector.tensor_tensor(eBm[0:64, :], eB[0:64, :],
                                            mskB[:], op=AL.mult)
                    yield
                    oA = ps_oA.tile([128, 260], f32, tag="oa")
                    oB = ps_gB.tile([64, 512], f32, tag="gb")
                    for wq in range(4):
                        wl = 4 * quad + wq
                        wg = 16 * half + wl
                        nc.tensor.matmul(oA[0:128, 65 * wq:65 * wq + 65],
                                         eAm[:, 128 * wq:128 * wq + 128],
                                         v3[:, 65 * wg:65 * wg + 65],
                                         start=True, stop=True)
                        nc.tensor.matmul(oB[0:64, 65 * wq:65 * wq + 65],
                                         eBm[0:64, 64 * wq:64 * wq + 64],
                                         v4[0:64, 65 * wg:65 * wg + 65],
                                         start=True, stop=True)
                    oAv = oA[0:128, :].rearrange("p (w c) -> p w c", w=4)
                    oBv = oB[0:64, 0:260].rearrange("p (w c) -> p w c", w=4)
                    rzA = sm_p.tile([128, 4], f32, tag="rzA")
                    rzB = sm_p.tile([64, 4], f32, tag="rzB")
                    nc.vector.reciprocal(
                        rzA[:].rearrange("p (w c) -> p w c", c=1),
                        oAv[:, :, 64:65])
                    nc.vector.reciprocal(
                        rzB[0:64, :].rearrange("p (w c) -> p w c", c=1),
                        oBv[:, :, 64:65])
                    ob = 1024 * half + 256 * quad
                    nc.vector.scalar_tensor_tensor(
                        att_A[:, ob:ob + 256].rearrange("p (w c) -> p w c", w=4),
                        oAv[:, :, 0:64],
                        1.0,
                        rzA[:].unsqueeze(2).broadcast_to((128, 4, 64)),
                        AL.mult, AL.mult)
                    nc.vector.scalar_tensor_tensor(
                        att_B[0:64, ob:ob + 256].rearrange("p (w c) -> p w c",
                                                           w=4),
                        oBv[:, :, 0:64],
                        1.0,
                        rzB[0:64, :].unsqueeze(2).broadcast_to((64, 4, 64)),
                        AL.mult, AL.mult)
                    yield

            # ---- proj GEMM (restores flat pixel order) ----
            yA = y_p.tile([128, 2048], bf, tag="yA")
            yB = y_p.tile([64, 2048], bf, tag="yB")
            attAr = att_A[:, :].rearrange("p (w r c) -> p r w c", w=32, r=8)
            attBr = att_B[0:64, :].rearrange("p (w r c) -> p r w c", w=32, r=8)
            for nbp in range(2):
                ppAs = [ps_big.tile([128, 512], f32, tag="big",
                                    name=f"ppA{nbp}_{i}") for i in range(2)]
                ppBs = [ps_gB.tile([64, 512], f32, tag="gb",
                                   name=f"ppB{nbp}_{i}") for i in range(2)]
                for mo2, msz2, pps in ((0, 128, ppAs), (128, 64, ppBs)):
                    for kc in range(2):
                        lhsT = (pjA, pjB)[kc][:, mo2:mo2 + msz2]
                        for i, pp in enumerate(pps):
                            nb = 2 * nbp + i
                            rhs = (attAr, attBr)[kc][:, 2 * nb:2 * nb + 2, :, :]
                            nc.tensor.matmul(pp[0:msz2, :], lhsT, rhs,
                                             start=(kc == 0), stop=(kc == 1))
                for i in range(2):
                    nb = 2 * nbp + i
                    nc.vector.tensor_copy(yA[:, 512 * nb:512 * (nb + 1)],
                                          ppAs[i][:])
                    nc.scalar.copy(yB[0:64, 512 * nb:512 * (nb + 1)],
                                   ppBs[i][0:64, :])
                yield
            nc.sync.dma_start(y_e[0:128, 8 * s:8 * s + 8, :], yA[:])
            nc.sync.dma_start(y_e[128:192, 8 * s:8 * s + 8, :], yB[0:64, :])

        # ---- interleaved emission: att(s-1) units between A/B(s) units ----
        for s in range(NSTRIP + 1):
            g_att = gen_att(s - 1) if s >= 1 else None
            g_ab = gen_AB(s) if s < NSTRIP else None
            while g_att is not None or g_ab is not None:
                if g_att is not None:
                    try:
                        next(g_att)
                    except StopIteration:
                        g_att = None
                if g_ab is not None:
                    try:
                        next(g_ab)
                    except StopIteration:
                        g_ab = None

    nc.finalize()
    return nc


def _host_prep(inputs):
    x = np.ascontiguousarray(np.asarray(inputs["x"], dtype=np.float32))
    qkv_w = np.asarray(inputs["qkv_w"], dtype=np.float32)[:, :, 0, 0]
    dw_w = np.asarray(inputs["dw_w"], dtype=np.float32)[:, 0]
    proj_w = np.asarray(inputs["proj_w"], dtype=np.float32)[:, :, 0, 0]
    temp = np.asarray(inputs["temperature"], dtype=np.float32)[:, 0, 0]

    qT = np.ascontiguousarray(qkv_w.T)  # [192, 576]
    wqA = qT[0:128, :].astype(BF16)
    wqB = np.ascontiguousarray(qT[128:192, :]).astype(BF16)

    MCH = [(0, 128), (128, 128), (256, 128), (384, 128), (512, 64)]
    dgA = np.zeros((128, 36 * 128), np.float32)
    dgB = np.zeros((64, 9 * 64), np.float32)
    for m, (mo, msz) in enumerate(MCH):
        for t in range(9):
            d = np.diag(dw_w[mo:mo + msz, t // 3, t % 3])
            if m < 4:
                dgA[0:msz, (9 * m + t) * msz:(9 * m + t + 1) * msz] = d
            else:
                dgB[0:msz, t * msz:(t + 1) * msz] = d
    dgA = dgA.astype(BF16)
    dgB = dgB.astype(BF16)

    projT = np.ascontiguousarray(proj_w.T)
    pjA = np.ascontiguousarray(projT[0:128, :]).astype(BF16)
    pjB = np.ascontiguousarray(projT[128:192, :]).astype(BF16)
    tmpA = np.repeat(temp[0:4], 32).reshape(128, 1).astype(np.float32)
    tmpB = np.repeat(temp[4:6], 32).reshape(64, 1).astype(np.float32)
    idn = np.eye(128, dtype=np.float32).astype(BF16)
    mskA = np.zeros((128, 512), np.float32)
    for g in range(4):
        for rep in range(4):
            mskA[32 * g:32 * g + 32,
                 128 * rep + 32 * g:128 * rep + 32 * g + 32] = 1.0
    mskB = np.zeros((64, 256), np.float32)
    for g in range(2):
        for rep in range(4):
            mskB[32 * g:32 * g + 32,
                 64 * rep + 32 * g:64 * rep + 32 * g + 32] = 1.0
    mskA = mskA.astype(BF16)
    mskB = mskB.astype(BF16)

    xpad = np.pad(x, ((0, 0), (0, 0), (1, 1), (1, 1)))
    in_maps = []
    for core in range(NCORES):
        b, halfc = core // 2, core % 2
        r0 = 128 * halfc
        xp = np.ascontiguousarray(xpad[b, :, r0:r0 + 130, :]).astype(BF16)
        in_maps.append(dict(xp=xp, wqA=wqA, wqB=wqB, dgA=dgA, dgB=dgB,
                            pjA=pjA, pjB=pjB, tmpA=tmpA, tmpB=tmpB, idn=idn,
                            mskA=mskA, mskB=mskB))
    return in_maps


def kernel(**inputs):
    global LAST_RESULT
    from concourse.bass_utils import run_bass_kernel_spmd

    in_maps = _host_prep(inputs)

    if "nc" not in _CACHE:
        _CACHE["nc"] = _build_nc()
    nc = _CACHE["nc"]

    res = run_bass_kernel_spmd(nc, in_maps, list(range(NCORES)))
    LAST_RESULT = res

    out = np.zeros((B, DIM, H, W), np.float32)
    for core in range(NCORES):
        b, halfc = core // 2, core % 2
        r0 = 128 * halfc
        out[b, :, r0:r0 + 128, :] = res.results[core]["y"].astype(np.float32)
    return out


# revision 21
# speedup vs baseline: 1.1696x; 1.1696x over previous
"""Trainium2 Bass kernel for nn_Channel_Attention (XCA-style windowed channel attention).

v3 = v2 + software pipelining: the attention phase of strip s-1 is emitted
interleaved with the qkv/depthwise GEMM stages of strip s (disjoint PSUM
banks, double-buffered qk/v tiles), so the PE never idles long enough for
HAM to re-throttle and DVE/ACT chains hide under PE work.

Kernel math (per core, data-parallel over batch x H-half):
  - stage A: qkv1 = conv1x1(x) on a 10-row halo strip (bf16 GEMM, K=192)
  - stage B: qkv_dw = sum_t diag(dw[:,t]) @ shift_t(qkv1)  (9 accumulating bf16
    matmuls per chunk, K = chunk size <= 128)
  - per-(chan,window) l2 norms: ACT Square + DVE window reduce + Ln + Exp(-0.5 ln)
  - q,k evacuated f32 with fused 1/norm scaling (DVE stt, broadcast AP)
  - per-window pc layout via PE transpose (f32: bf16 PSUM is broken on trn2 HW)
  - transposed gram G2[j,i] with k zero-padded to K=128 (bf16 accum groups with
    mixed tile positions crash trn2; all matmuls full-K at position (0,0))
  - exp feeds out-matmul as lhsT directly; ones column in v gives Z in-matmul
  - 1/Z fused into evacuation; final proj GEMM restores flat pixel order
"""
import sys

sys.path.insert(0, "/opt/trn_rl_repo")

import numpy as np
import ml_dtypes

BF16 = ml_dtypes.bfloat16

DIM, HEADS, PS = 192, 6, 8
B, H, W = 4, 256, 256
C3 = 3 * DIM  # 576
NCORES = 8
ROWS = 128
NSTRIP = 16

_CACHE = {}
LAST_RESULT = None


def _build_nc():
    import contextlib

    import concourse.mybir as mybir
    import concourse.tile as tile
    from concourse import bacc

    f32 = mybir.dt.float32
    bf = mybir.dt.bfloat16
    AL = mybir.AluOpType
    AF = mybir.ActivationFunctionType
    AX = mybir.AxisListType

    nc = bacc.Bacc("TRN2", target_bir_lowering=False, debug=False, num_devices=NCORES)

    xp_e = nc.declare_dram_parameter("xp", [DIM, 130, 258], bf, isOutput=False)
    wqA_e = nc.declare_dram_parameter("wqA", [128, C3], bf, isOutput=False)
    wqB_e = nc.declare_dram_parameter("wqB", [64, C3], bf, isOutput=False)
    dgA_e = nc.declare_dram_parameter("dgA", [128, 36 * 128], bf, isOutput=False)
    dgB_e = nc.declare_dram_parameter("dgB", [64, 9 * 64], bf, isOutput=False)
    pjA_e = nc.declare_dram_parameter("pjA", [128, 192], bf, isOutput=False)
    pjB_e = nc.declare_dram_parameter("pjB", [64, 192], bf, isOutput=False)
    tmpA_e = nc.declare_dram_parameter("tmpA", [128, 1], f32, isOutput=False)
    tmpB_e = nc.declare_dram_parameter("tmpB", [64, 1], f32, isOutput=False)
    idn_e = nc.declare_dram_parameter("idn", [128, 128], bf, isOutput=False)
    mskA_e = nc.declare_dram_parameter("mskA", [128, 512], bf, isOutput=False)
    mskB_e = nc.declare_dram_parameter("mskB", [64, 256], bf, isOutput=False)
    y_e = nc.declare_dram_parameter("y", [DIM, ROWS, 256], bf, isOutput=True)

    with tile.TileContext(nc) as tc, contextlib.ExitStack() as ctx:
        const = ctx.enter_context(tc.tile_pool(name="const", bufs=1))
        xs_p = ctx.enter_context(tc.tile_pool(name="xs", bufs=2))
        q1_p = ctx.enter_context(tc.tile_pool(name="q1", bufs=1))
        qk_p = ctx.enter_context(tc.tile_pool(name="qk", bufs=2))
        pc_p = ctx.enter_context(tc.tile_pool(name="pc", bufs=1))
        sm_p = ctx.enter_context(tc.tile_pool(name="small", bufs=2))
        stg_p = ctx.enter_context(tc.tile_pool(name="stg", bufs=2))
        e_p = ctx.enter_context(tc.tile_pool(name="eb", bufs=2))
        att_p = ctx.enter_context(tc.tile_pool(name="att", bufs=1))
        y_p = ctx.enter_context(tc.tile_pool(name="yb", bufs=1))
        ps_big = ctx.enter_context(tc.tile_pool(name="psbig", bufs=4, space="PSUM"))
        ps_pct = ctx.enter_context(tc.tile_pool(name="pspct", bufs=1, space="PSUM"))
        ps_gA = ctx.enter_context(tc.tile_pool(name="psga", bufs=1, space="PSUM"))
        ps_gB = ctx.enter_context(tc.tile_pool(name="psgb", bufs=1, space="PSUM"))
        ps_oA = ctx.enter_context(tc.tile_pool(name="psoa", bufs=1, space="PSUM"))

        # ---- constants ----
        wqA = const.tile([128, C3], bf)
        nc.sync.dma_start(wqA[:], wqA_e[:, :])
        wqB = const.tile([64, C3], bf)
        nc.sync.dma_start(wqB[:], wqB_e[:, :])
        dgA = const.tile([128, 36 * 128], bf)
        nc.sync.dma_start(dgA[:], dgA_e[:, :])
        dgB = const.tile([64, 9 * 64], bf)
        nc.sync.dma_start(dgB[:], dgB_e[:, :])
        pjA = const.tile([128, 192], bf)
        nc.sync.dma_start(pjA[:], pjA_e[:, :])
        pjB = const.tile([64, 192], bf)
        nc.sync.dma_start(pjB[:], pjB_e[:, :])
        tmpA = const.tile([128, 1], f32)
        nc.sync.dma_start(tmpA[:], tmpA_e[:, :])
        tmpB = const.tile([64, 1], f32)
        nc.sync.dma_start(tmpB[:], tmpB_e[:, :])
        idn = const.tile([128, 128], bf)
        nc.sync.dma_start(idn[:], idn_e[:, :])
        mskA = const.tile([128, 512], bf)
        nc.sync.dma_start(mskA[:], mskA_e[:, :])
        mskB = const.tile([64, 256], bf)
        nc.sync.dma_start(mskB[:], mskB_e[:, :])

        MCH = [(0, 128), (128, 128), (256, 128), (384, 128), (512, 64)]

        # k in pc layout, zero-padded to full 128 partitions per window parity
        kzE = pc_p.tile([128, 8 * 192], bf, name="kzE")
        kzO = pc_p.tile([128, 8 * 192], bf, name="kzO")
        nc.vector.memset(kzE[64:128, :], 0.0)
        nc.vector.memset(kzO[0:64, :], 0.0)

        # qkv1 (pre-depthwise) chunk tiles: [chans, 10 rows x 258 cols]
        q1 = [q1_p.tile([128, 2580], bf, name=f"q1c{m}") for m in range(4)]
        q1.append(q1_p.tile([64, 2580], bf, name="q1c4"))
        q1v = [t[0:msz, :].rearrange("p (r c) -> p r c", r=10)
               for t, (mo, msz) in zip(q1, MCH)]

        state = {}  # per-strip tiles handed from gen_AB(s) to gen_att(s)

        def gen_AB(s):
            # ---- load x strip (10 rows x 258 cols, padded) ----
            xsA = xs_p.tile([128, 10 * 258], bf, tag="xsA")
            nc.sync.dma_start(xsA[:], xp_e[0:128, 8 * s:8 * s + 10, :])
            xsB = xs_p.tile([64, 10 * 258], bf, tag="xsB")
            nc.sync.dma_start(xsB[:], xp_e[128:192, 8 * s:8 * s + 10, :])

            qk = [qk_p.tile([128, 2048], bf, name=f"qk{m}", tag=f"qk{m}")
                  for m in range(3)]
            # v tiles: 65-col window stride, col 64 of each window = 1.0 so
            # out-matmuls against [v | 1] produce Z in the extra column.
            v3 = qk_p.tile([128, 32 * 65], bf, tag="v3")
            v4 = qk_p.tile([64, 32 * 65], bf, tag="v4")
            v3w = v3[:].rearrange("p (w c) -> p w c", w=32)
            v4w = v4[0:64, :].rearrange("p (w c) -> p w c", w=32)
            nc.vector.memset(v3w[:, :, 64:65], 1.0)
            nc.vector.memset(v4w[:, :, 64:65], 1.0)
            state[s] = (qk, v3, v4)

            # ---- stage A: qkv1 = conv1x1(x) over the 10-row halo strip ----
            for m, (mo, msz) in enumerate(MCH):
                for ng in range(3):
                    psq = [ps_big.tile([128, 512], f32, tag="big",
                                       name=f"psq{m}_{ng}_{i}")
                           for i in range(2)]
                    for kc in range(2):
                        wq = (wqA, wqB)[kc]
                        xf = (xsA, xsB)[kc]
                        for nb, ps in enumerate(psq):
                            c0 = (2 * ng + nb) * 430
                            nc.tensor.matmul(
                                ps[0:msz, 0:430], wq[:, mo:mo + msz],
                                xf[:, c0:c0 + 430],
                                start=(kc == 0), stop=(kc == 1))
                    for nb, ps in enumerate(psq):
                        c0 = (2 * ng + nb) * 430
                        if nb == 0 and ng != 1:
                            nc.vector.tensor_copy(q1[m][0:msz, c0:c0 + 430],
                                                  ps[0:msz, 0:430])
                        else:
                            nc.scalar.copy(q1[m][0:msz, c0:c0 + 430],
                                           ps[0:msz, 0:430])
                    yield

            # ---- stage B: depthwise via diag matmuls + norms + evac ----
            # N-block j = 8 rows x cols [64j, 64j+64) = windows [8j, 8j+8)
            for m, (mo, msz) in enumerate(MCH):
                dg, dbase = (dgA, 9 * m) if m < 4 else (dgB, 0)
                psl = []
                for j in range(4):
                    ps = ps_big.tile([128, 512], f32, tag="big",
                                     name=f"psb{m}_{j}")
                    psl.append((j, ps))
                for t in range(9):
                    dy, dx = t // 3, t % 3
                    lhsT = dg[:, (dbase + t) * msz:(dbase + t + 1) * msz]
                    for j, ps in psl:
                        rhs = q1v[m][:, dy:dy + 8,
                                     64 * j + dx:64 * j + dx + 64
                                     ].rearrange("p r (w c) -> p w r c", w=8)
                        nc.tensor.matmul(
                            ps[0:msz, :], lhsT, rhs,
                            start=(t == 0), stop=(t == 8))
                if m < 3:
                    # fast staging copy frees the PSUM banks immediately;
                    # norms + scaled evac then read SBUF
                    stg = stg_p.tile([128, 2048], bf, tag="stg")
                    for j, ps in psl:
                        if j < 2:
                            nc.vector.tensor_copy(
                                stg[0:msz, 512 * j:512 * (j + 1)], ps[0:msz, :])
                        else:
                            nc.scalar.copy(
                                stg[0:msz, 512 * j:512 * (j + 1)], ps[0:msz, :])
                    yield
                    n2 = sm_p.tile([128, 32], f32, tag="n2")
                    for j, ps in psl:
                        sq = sm_p.tile([128, 512], f32, tag="sq")
                        nc.scalar.activation(sq[0:msz, :],
                                             stg[0:msz, 512 * j:512 * (j + 1)],
                                             AF.Square)
                        sqv = sq[0:msz, :].rearrange("p (w c) -> p w c", w=8)
                        nc.vector.tensor_reduce(
                            n2[0:msz, 8 * j:8 * j + 8], sqv,
                            axis=AX.X, op=AL.add)
                    yield
                    lnn = sm_p.tile([128, 32], f32, tag="nrm")
                    nc.scalar.activation(lnn[0:msz, :], n2[0:msz, :],
                                         AF.Ln)
                    rj = sm_p.tile([128, 32], f32, tag="rj")
                    nc.scalar.activation(rj[0:msz, :], lnn[0:msz, :],
                                         AF.Exp, scale=-0.5)
                    for j, ps in psl:
                        # evac with 1/norm scaling (win-grouped layout)
                        dst = qk[m][:, 512 * j:512 * (j + 1)].rearrange(
                            "p (w c) -> p w c", w=8)
                        nc.vector.scalar_tensor_tensor(
                            dst,
                            stg[0:msz, 512 * j:512 * (j + 1)].rearrange(
                                "p (w c) -> p w c", w=8),
                            1.0,
                            rj[0:msz, 8 * j:8 * j + 8].unsqueeze(2)
                            .broadcast_to((msz, 8, 64)),
                            AL.mult, AL.mult)
                    yield
                else:
                    for j, ps in psl:
                        vw = (v3w, v4w)[m - 3]
                        nc.scalar.copy(
                            vw[0:msz, 8 * j:8 * j + 8, 0:64],
                            ps[0:msz, :].rearrange("p (w c) -> p w c", w=8))
                    yield

        def gen_att(s):
            qk, v3, v4 = state.pop(s)
            # ---- pc transposes + attention, halves of 16 windows ----
            for half in range(2):
                if half == 0:
                    att_A = att_p.tile([128, 2048], bf, tag="attA")
                    att_B = att_p.tile([64, 2048], bf, tag="attB")
                qkpc = pc_p.tile([128, 8 * 192], bf, tag="qkpc")
                for p in range(8):
                    w0 = 16 * half + 2 * p
                    pcps = ps_pct.tile([128, 384], f32, tag="pct")
                    for (srct, oc) in ((qk[0], 0), (qk[1], 128), (qk[2], 256)):
                        inap = srct[0:128, 64 * w0:64 * w0 + 128]
                        # transpose as a regular bf16 matmul: in^T @ I
                        nc.tensor.matmul(pcps[:, oc:oc + 128], inap, idn[:, :],
                                         start=True, stop=True,
                                         skip_group_check=True)
                    nc.vector.tensor_copy(qkpc[:, 192 * p:192 * (p + 1)],
                                          pcps[:, 0:192])
                    nc.vector.tensor_copy(kzE[0:64, 192 * p:192 * (p + 1)],
                                          pcps[0:64, 192:384])
                    nc.scalar.copy(kzO[64:128, 192 * p:192 * (p + 1)],
                                   pcps[64:128, 192:384])
                    if p % 2 == 1:
                        yield
                # grams (transposed: G2[j,i]) + attention, quarters of 4 windows
                for quad in range(4):
                    gA = ps_gA.tile([128, 512], f32, tag="ga")
                    gB = ps_gB.tile([64, 512], f32, tag="gb")
                    for wq in range(4):
                        wl = 4 * quad + wq
                        p, wsub = wl // 2, wl % 2
                        kz = (kzE, kzO)[wsub]
                        qa = qkpc[0:128, 192 * p:192 * p + 128]
                        ka = kz[0:128, 192 * p:192 * p + 128]
                        nc.tensor.matmul(gA[:, 128 * wq:128 * wq + 128], ka, qa,
                                         start=(wq == 0), stop=(wq == 3),
                                         skip_group_check=True)
                        qb = qkpc[0:128, 192 * p + 128:192 * p + 192]
                        kb = kz[0:128, 192 * p + 128:192 * p + 192]
                        nc.tensor.matmul(gB[0:64, 64 * wq:64 * wq + 64], kb, qb,
                                         start=(wq == 0), stop=(wq == 3),
                                         skip_group_check=True)
                    eA = e_p.tile([128, 512], bf, tag="eA")
                    eB = e_p.tile([64, 256], bf, tag="eB")
                    nc.scalar.activation(eA[:], gA[:], AF.Exp, scale=tmpA[:])
                    nc.scalar.activation(eB[0:64, :], gB[0:64, 0:256], AF.Exp,
                                         scale=tmpB[:])
                    eAm = e_p.tile([128, 512], bf, tag="eAm")
                    eBm = e_p.tile([64, 256], bf, tag="eBm")
                    nc.vector.tensor_tensor(eAm[:], eA[:], mskA[:], op=AL.mult)
                    nc.vector.tensor_tensor(eBm[0:64, :], eB[0:64, :],
                                            mskB[:], op=AL.mult)
                    yield
                    oA = ps_oA.tile([128, 260], f32, tag="oa")
                    oB = ps_gB.tile([64, 512], f32, tag="gb")
                    for wq in range(4):
                        wl = 4 * quad + wq
                        wg = 16 * half + wl
                        nc.tensor.matmul(oA[0:128, 65 * wq:65 * wq + 65],
                                         eAm[:, 128 * wq:128 * wq + 128],
                                         v3[:, 65 * wg:65 * wg + 65],
                                         start=True, stop=True)
                        nc.tensor.matmul(oB[0:64, 65 * wq:65 * wq + 65],
                                         eBm[0:64, 64 * wq:64 * wq + 64],
                                         v4[0:64, 65 * wg:65 * wg + 65],
                                         start=True, stop=True)
                    oAv = oA[0:128, :].rearrange("p (w c) -> p w c", w=4)
                    oBv = oB[0:64, 0:260].rearrange("p (w c) -> p w c", w=4)
                    rzA = sm_p.tile([128, 4], f32, tag="rzA")
                    rzB = sm_p.tile([64, 4], f32, tag="rzB")
                    nc.vector.reciprocal(
                        rzA[:].rearrange("p (w c) -> p w c", c=1),
                        oAv[:, :, 64:65])
                    nc.vector.reciprocal(
                        rzB[0:64, :].rearrange("p (w c) -> p w c", c=1),
                        oBv[:, :, 64:65])
                    ob = 1024 * half + 256 * quad
                    nc.vector.scalar_tensor_tensor(
                        att_A[:, ob:ob + 256].rearrange("p (w c) -> p w c", w=4),
                        oAv[:, :, 0:64],
                        1.0,
                        rzA[:].unsqueeze(2).broadcast_to((128, 4, 64)),
                        AL.mult, AL.mult)
                    nc.vector.scalar_tensor_tensor(
                        att_B[0:64, ob:ob + 256].rearrange("p (w c) -> p w c",
                                                           w=4),
                        oBv[:, :, 0:64],
                        1.0,
                        rzB[0:64, :].unsqueeze(2).broadcast_to((64, 4, 64)),
                        AL.mult, AL.mult)
                    yield

            # ---- proj GEMM (restores flat pixel order) ----
            yA = y_p.tile([128, 2048], bf, tag="yA")
            yB = y_p.tile([64, 2048], bf, tag="yB")
            attAr = att_A[:, :].rearrange("p (w r c) -> p r w c", w=32, r=8)
            attBr = att_B[0:64, :].rearrange("p (w r c) -> p r w c", w=32, r=8)
            for nbp in range(2):
                ppAs = [ps_big.tile([128, 512], f32, tag="big",
                                    name=f"ppA{nbp}_{i}") for i in range(2)]
                ppBs = [ps_gB.tile([64, 512], f32, tag="gb",
                                   name=f"ppB{nbp}_{i}") for i in range(2)]
                for mo2, msz2, pps in ((0, 128, ppAs), (128, 64, ppBs)):
                    for kc in range(2):
                        lhsT = (pjA, pjB)[kc][:, mo2:mo2 + msz2]
                        for i, pp in enumerate(pps):
                            nb = 2 * nbp + i
                            rhs = (attAr, attBr)[kc][:, 2 * nb:2 * nb + 2, :, :]
                            nc.tensor.matmul(pp[0:msz2, :], lhsT, rhs,
                                             start=(kc == 0), stop=(kc == 1))
                for i in range(2):
                    nb = 2 * nbp + i
                    nc.vector.tensor_copy(yA[:, 512 * nb:512 * (nb + 1)],
                                          ppAs[i][:])
                    nc.scalar.copy(yB[0:64, 512 * nb:512 * (nb + 1)],
                                   ppBs[i][0:64, :])
                yield
            nc.sync.dma_start(y_e[0:128, 8 * s:8 * s + 8, :], yA[:])
            nc.sync.dma_start(y_e[128:192, 8 * s:8 * s + 8, :], yB[0:64, :])

        # ---- interleaved emission: att(s-1) units between A/B(s) units ----
        for s in range(NSTRIP + 1):
            g_att = gen_att(s - 1) if s >= 1 else None
            g_ab = gen_AB(s) if s < NSTRIP else None
            while g_att is not None or g_ab is not None:
                if g_att is not None:
                    try:
                        next(g_att)
                    except StopIteration:
                        g_att = None
                if g_ab is not None:
                    try:
                        next(g_ab)
                    except StopIteration:
                        g_ab = None

    nc.finalize()
    return nc


def _host_prep(inputs):
    x = np.ascontiguousarray(np.asarray(inputs["x"], dtype=np.float32))
    qkv_w = np.asarray(inputs["qkv_w"], dtype=np.float32)[:, :, 0, 0]
    dw_w = np.asarray(inputs["dw_w"], dtype=np.float32)[:, 0]
    proj_w = np.asarray(inputs["proj_w"], dtype=np.float32)[:, :, 0, 0]
    temp = np.asarray(inputs["temperature"], dtype=np.float32)[:, 0, 0]

    qT = np.ascontiguousarray(qkv_w.T)  # [192, 576]
    wqA = qT[0:128, :].astype(BF16)
    wqB = np.ascontiguousarray(qT[128:192, :]).astype(BF16)

    MCH = [(0, 128), (128, 128), (256, 128), (384, 128), (512, 64)]
    dgA = np.zeros((128, 36 * 128), np.float32)
    dgB = np.zeros((64, 9 * 64), np.float32)
    for m, (mo, msz) in enumerate(MCH):
        for t in range(9):
            d = np.diag(dw_w[mo:mo + msz, t // 3, t % 3])
            if m < 4:
                dgA[0:msz, (9 * m + t) * msz:(9 * m + t + 1) * msz] = d
            else:
                dgB[0:msz, t * msz:(t + 1) * msz] = d
    dgA = dgA.astype(BF16)
    dgB = dgB.astype(BF16)

    projT = np.ascontiguousarray(proj_w.T)
    pjA = np.ascontiguousarray(projT[0:128, :]).astype(BF16)
    pjB = np.ascontiguousarray(projT[128:192, :]).astype(BF16)
    tmpA = np.repeat(temp[0:4], 32).reshape(128, 1).astype(np.float32)
    tmpB = np.repeat(temp[4:6], 32).reshape(64, 1).astype(np.float32)
    idn = np.eye(128, dtype=np.float32).astype(BF16)
    mskA = np.zeros((128, 512), np.float32)
    for g in range(4):
        for rep in range(4):
            mskA[32 * g:32 * g + 32,
                 128 * rep + 32 * g:128 * rep + 32 * g + 32] = 1.0
    mskB = np.zeros((64, 256), np.float32)
    for g in range(2):
        for rep in range(4):
            mskB[32 * g:32 * g + 32,
                 64 * rep + 32 * g:64 * rep + 32 * g + 32] = 1.0
    mskA = mskA.astype(BF16)
    mskB = mskB.astype(BF16)

    xpad = np.pad(x, ((0, 0), (0, 0), (1, 1), (1, 1)))
    in_maps = []
    for core in range(NCORES):
        b, halfc = core // 2, core % 2
        r0 = 128 * halfc
        xp = np.ascontiguousarray(xpad[b, :, r0:r0 + 130, :]).astype(BF16)
        in_maps.append(dict(xp=xp, wqA=wqA, wqB=wqB, dgA=dgA, dgB=dgB,
                            pjA=pjA, pjB=pjB, tmpA=tmpA, tmpB=tmpB, idn=idn,
                            mskA=mskA, mskB=mskB))
    return in_maps


def kernel(**inputs):
    global LAST_RESULT
    from concourse.bass_utils import run_bass_kernel_spmd

    in_maps = _host_prep(inputs)

    if "nc" not in _CACHE:
        _CACHE["nc"] = _build_nc()
    nc = _CACHE["nc"]

    res = run_bass_kernel_spmd(nc, in_maps, list(range(NCORES)))
    LAST_RESULT = res

    out = np.zeros((B, DIM, H, W), np.float32)
    for core in range(NCORES):
        b, halfc = core // 2, core % 2
        r0 = 128 * halfc
        out[b, :, r0:r0 + 128, :] = res.results[core]["y"].astype(np.float32)
    return out


# revision 22
# speedup vs baseline: 1.1983x; 1.0246x over previous
"""Trainium2 Bass kernel for nn_Channel_Attention (XCA-style windowed channel attention).

v3 = v2 + software pipelining: the attention phase of strip s-1 is emitted
interleaved with the qkv/depthwise GEMM stages of strip s (disjoint PSUM
banks, double-buffered qk/v tiles), so the PE never idles long enough for
HAM to re-throttle and DVE/ACT chains hide under PE work.

Kernel math (per core, data-parallel over batch x H-half):
  - stage A: qkv1 = conv1x1(x) on a 10-row halo strip (bf16 GEMM, K=192)
  - stage B: qkv_dw = sum_t diag(dw[:,t]) @ shift_t(qkv1)  (9 accumulating bf16
    matmuls per chunk, K = chunk size <= 128)
  - per-(chan,window) l2 norms: ACT Square + DVE window reduce + Ln + Exp(-0.5 ln)
  - q,k evacuated f32 with fused 1/norm scaling (DVE stt, broadcast AP)
  - per-window pc layout via PE transpose (f32: bf16 PSUM is broken on trn2 HW)
  - transposed gram G2[j,i] with k zero-padded to K=128 (bf16 accum groups with
    mixed tile positions crash trn2; all matmuls full-K at position (0,0))
  - exp feeds out-matmul as lhsT directly; ones column in v gives Z in-matmul
  - 1/Z fused into evacuation; final proj GEMM restores flat pixel order
"""
import sys

sys.path.insert(0, "/opt/trn_rl_repo")

import numpy as np
import ml_dtypes

BF16 = ml_dtypes.bfloat16

DIM, HEADS, PS = 192, 6, 8
B, H, W = 4, 256, 256
C3 = 3 * DIM  # 576
NCORES = 8
ROWS = 128
NSTRIP = 16

_CACHE = {}
LAST_RESULT = None


def _build_nc():
    import contextlib

    import concourse.mybir as mybir
    import concourse.tile as tile
    from concourse import bacc

    f32 = mybir.dt.float32
    bf = mybir.dt.bfloat16
    AL = mybir.AluOpType
    AF = mybir.ActivationFunctionType
    AX = mybir.AxisListType

    nc = bacc.Bacc("TRN2", target_bir_lowering=False, debug=False, num_devices=NCORES)

    xp_e = nc.declare_dram_parameter("xp", [DIM, 130, 258], bf, isOutput=False)
    wqA_e = nc.declare_dram_parameter("wqA", [128, C3], bf, isOutput=False)
    wqB_e = nc.declare_dram_parameter("wqB", [64, C3], bf, isOutput=False)
    dgA_e = nc.declare_dram_parameter("dgA", [128, 36 * 128], bf, isOutput=False)
    dgB_e = nc.declare_dram_parameter("dgB", [64, 9 * 64], bf, isOutput=False)
    pjA_e = nc.declare_dram_parameter("pjA", [128, 192], bf, isOutput=False)
    pjB_e = nc.declare_dram_parameter("pjB", [64, 192], bf, isOutput=False)
    tmpA_e = nc.declare_dram_parameter("tmpA", [128, 1], f32, isOutput=False)
    tmpB_e = nc.declare_dram_parameter("tmpB", [64, 1], f32, isOutput=False)
    idn_e = nc.declare_dram_parameter("idn", [128, 128], bf, isOutput=False)
    mskA_e = nc.declare_dram_parameter("mskA", [128, 512], bf, isOutput=False)
    mskB_e = nc.declare_dram_parameter("mskB", [64, 256], bf, isOutput=False)
    y_e = nc.declare_dram_parameter("y", [DIM, ROWS, 256], bf, isOutput=True)

    with tile.TileContext(nc) as tc, contextlib.ExitStack() as ctx:
        const = ctx.enter_context(tc.tile_pool(name="const", bufs=1))
        xs_p = ctx.enter_context(tc.tile_pool(name="xs", bufs=2))
        q1_p = ctx.enter_context(tc.tile_pool(name="q1", bufs=1))
        qk_p = ctx.enter_context(tc.tile_pool(name="qk", bufs=2))
        pc_p = ctx.enter_context(tc.tile_pool(name="pc", bufs=1))
        sm_p = ctx.enter_context(tc.tile_pool(name="small", bufs=2))
        stg_p = ctx.enter_context(tc.tile_pool(name="stg", bufs=2))
        e_p = ctx.enter_context(tc.tile_pool(name="eb", bufs=2))
        att_p = ctx.enter_context(tc.tile_pool(name="att", bufs=1))
        y_p = ctx.enter_context(tc.tile_pool(name="yb", bufs=1))
        ps_big = ctx.enter_context(tc.tile_pool(name="psbig", bufs=4, space="PSUM"))
        ps_pct = ctx.enter_context(tc.tile_pool(name="pspct", bufs=1, space="PSUM"))
        ps_gA = ctx.enter_context(tc.tile_pool(name="psga", bufs=1, space="PSUM"))
        ps_gB = ctx.enter_context(tc.tile_pool(name="psgb", bufs=1, space="PSUM"))
        ps_oA = ctx.enter_context(tc.tile_pool(name="psoa", bufs=1, space="PSUM"))

        # ---- constants ----
        wqA = const.tile([128, C3], bf)
        nc.sync.dma_start(wqA[:], wqA_e[:, :])
        wqB = const.tile([64, C3], bf)
        nc.sync.dma_start(wqB[:], wqB_e[:, :])
        dgA = const.tile([128, 36 * 128], bf)
        nc.sync.dma_start(dgA[:], dgA_e[:, :])
        dgB = const.tile([64, 9 * 64], bf)
        nc.sync.dma_start(dgB[:], dgB_e[:, :])
        pjA = const.tile([128, 192], bf)
        nc.sync.dma_start(pjA[:], pjA_e[:, :])
        pjB = const.tile([64, 192], bf)
        nc.sync.dma_start(pjB[:], pjB_e[:, :])
        tmpA = const.tile([128, 1], f32)
        nc.sync.dma_start(tmpA[:], tmpA_e[:, :])
        tmpB = const.tile([64, 1], f32)
        nc.sync.dma_start(tmpB[:], tmpB_e[:, :])
        idn = const.tile([128, 128], bf)
        nc.sync.dma_start(idn[:], idn_e[:, :])
        mskA = const.tile([128, 512], bf)
        nc.sync.dma_start(mskA[:], mskA_e[:, :])
        mskB = const.tile([64, 256], bf)
        nc.sync.dma_start(mskB[:], mskB_e[:, :])

        MCH = [(0, 128), (128, 128), (256, 128), (384, 128), (512, 64)]

        # k in pc layout, zero-padded to full 128 partitions per window parity
        kzE = pc_p.tile([128, 8 * 192], bf, name="kzE")
        kzO = pc_p.tile([128, 8 * 192], bf, name="kzO")
        nc.vector.memset(kzE[64:128, :], 0.0)
        nc.vector.memset(kzO[0:64, :], 0.0)

        # qkv1 (pre-depthwise) chunk tiles: [chans, 10 rows x 258 cols]
        q1 = [q1_p.tile([128, 2580], bf, name=f"q1c{m}") for m in range(4)]
        q1.append(q1_p.tile([64, 2580], bf, name="q1c4"))
        q1v = [t[0:msz, :].rearrange("p (r c) -> p r c", r=10)
               for t, (mo, msz) in zip(q1, MCH)]

        state = {}  # per-strip tiles handed from gen_AB(s) to gen_att(s)

        def gen_AB(s):
            # ---- load x strip (10 rows x 258 cols, padded) ----
            xsA = xs_p.tile([128, 10 * 258], bf, tag="xsA")
            nc.sync.dma_start(xsA[:], xp_e[0:128, 8 * s:8 * s + 10, :])
            xsB = xs_p.tile([64, 10 * 258], bf, tag="xsB")
            nc.sync.dma_start(xsB[:], xp_e[128:192, 8 * s:8 * s + 10, :])

            qk = [qk_p.tile([128, 2048], bf, name=f"qk{m}", tag=f"qk{m}")
                  for m in range(3)]
            # v tiles: 65-col window stride, col 64 of each window = 1.0 so
            # out-matmuls against [v | 1] produce Z in the extra column.
            v3 = qk_p.tile([128, 32 * 65], bf, tag="v3")
            v4 = qk_p.tile([64, 32 * 65], bf, tag="v4")
            v3w = v3[:].rearrange("p (w c) -> p w c", w=32)
            v4w = v4[0:64, :].rearrange("p (w c) -> p w c", w=32)
            nc.vector.memset(v3w[:, :, 64:65], 1.0)
            nc.vector.memset(v4w[:, :, 64:65], 1.0)
            state[s] = (qk, v3, v4)

            # ---- stage A: qkv1 = conv1x1(x) ----
            # Strip 0 computes the full 10-row halo strip; later strips reuse
            # the previous strip's last 2 rows (q1 rows 8,9 == this strip's
            # rows 0,1) via an SBUF copy and compute only the 8 new rows.
            if s == 0:
                blocks = [(i * 430, 430) for i in range(6)]
            else:
                for m, (mo, msz) in enumerate(MCH):
                    nc.vector.tensor_copy(q1[m][0:msz, 0:516],
                                          q1[m][0:msz, 2064:2580])
                blocks = [(516 + i * 344, 344) for i in range(6)]
            for m, (mo, msz) in enumerate(MCH):
                for ng in range(3):
                    psq = [ps_big.tile([128, 512], f32, tag="big",
                                       name=f"psq{m}_{ng}_{i}")
                           for i in range(2)]
                    for kc in range(2):
                        wq = (wqA, wqB)[kc]
                        xf = (xsA, xsB)[kc]
                        for nb, ps in enumerate(psq):
                            c0, wdt = blocks[2 * ng + nb]
                            nc.tensor.matmul(
                                ps[0:msz, 0:wdt], wq[:, mo:mo + msz],
                                xf[:, c0:c0 + wdt],
                                start=(kc == 0), stop=(kc == 1))
                    for nb, ps in enumerate(psq):
                        c0, wdt = blocks[2 * ng + nb]
                        if nb == 0 and ng != 1:
                            nc.vector.tensor_copy(q1[m][0:msz, c0:c0 + wdt],
                                                  ps[0:msz, 0:wdt])
                        else:
                            nc.scalar.copy(q1[m][0:msz, c0:c0 + wdt],
                                           ps[0:msz, 0:wdt])
                    yield

            # ---- stage B: depthwise via diag matmuls + norms + evac ----
            # N-block j = 8 rows x cols [64j, 64j+64) = windows [8j, 8j+8)
            for m, (mo, msz) in enumerate(MCH):
                dg, dbase = (dgA, 9 * m) if m < 4 else (dgB, 0)
                psl = []
                for j in range(4):
                    ps = ps_big.tile([128, 512], f32, tag="big",
                                     name=f"psb{m}_{j}")
                    psl.append((j, ps))
                for t in range(9):
                    dy, dx = t // 3, t % 3
                    lhsT = dg[:, (dbase + t) * msz:(dbase + t + 1) * msz]
                    for j, ps in psl:
                        rhs = q1v[m][:, dy:dy + 8,
                                     64 * j + dx:64 * j + dx + 64
                                     ].rearrange("p r (w c) -> p w r c", w=8)
                        nc.tensor.matmul(
                            ps[0:msz, :], lhsT, rhs,
                            start=(t == 0), stop=(t == 8))
                if m < 3:
                    # fast staging copy frees the PSUM banks immediately;
                    # norms + scaled evac then read SBUF
                    stg = stg_p.tile([128, 2048], bf, tag="stg")
                    for j, ps in psl:
                        if j < 2:
                            nc.vector.tensor_copy(
                                stg[0:msz, 512 * j:512 * (j + 1)], ps[0:msz, :])
                        else:
                            nc.scalar.copy(
                                stg[0:msz, 512 * j:512 * (j + 1)], ps[0:msz, :])
                    yield
                    n2 = sm_p.tile([128, 32], f32, tag="n2")
                    for j, ps in psl:
                        sq = sm_p.tile([128, 512], f32, tag="sq")
                        nc.scalar.activation(sq[0:msz, :],
                                             stg[0:msz, 512 * j:512 * (j + 1)],
                                             AF.Square)
                        sqv = sq[0:msz, :].rearrange("p (w c) -> p w c", w=8)
                        nc.vector.tensor_reduce(
                            n2[0:msz, 8 * j:8 * j + 8], sqv,
                            axis=AX.X, op=AL.add)
                    yield
                    lnn = sm_p.tile([128, 32], f32, tag="nrm")
                    nc.scalar.activation(lnn[0:msz, :], n2[0:msz, :],
                                         AF.Ln)
                    rj = sm_p.tile([128, 32], f32, tag="rj")
                    nc.scalar.activation(rj[0:msz, :], lnn[0:msz, :],
                                         AF.Exp, scale=-0.5)
                    for j, ps in psl:
                        # evac with 1/norm scaling (win-grouped layout)
                        dst = qk[m][:, 512 * j:512 * (j + 1)].rearrange(
                            "p (w c) -> p w c", w=8)
                        nc.vector.scalar_tensor_tensor(
                            dst,
                            stg[0:msz, 512 * j:512 * (j + 1)].rearrange(
                                "p (w c) -> p w c", w=8),
                            1.0,
                            rj[0:msz, 8 * j:8 * j + 8].unsqueeze(2)
                            .broadcast_to((msz, 8, 64)),
                            AL.mult, AL.mult)
                    yield
                else:
                    for j, ps in psl:
                        vw = (v3w, v4w)[m - 3]
                        nc.scalar.copy(
                            vw[0:msz, 8 * j:8 * j + 8, 0:64],
                            ps[0:msz, :].rearrange("p (w c) -> p w c", w=8))
                    yield

        def gen_att(s):
            qk, v3, v4 = state.pop(s)
            # ---- pc transposes + attention, halves of 16 windows ----
            for half in range(2):
                if half == 0:
                    att_A = att_p.tile([128, 2048], bf, tag="attA")
                    att_B = att_p.tile([64, 2048], bf, tag="attB")
                qkpc = pc_p.tile([128, 8 * 192], bf, tag="qkpc")
                for p in range(8):
                    w0 = 16 * half + 2 * p
                    pcps = ps_pct.tile([128, 384], f32, tag="pct")
                    for (srct, oc) in ((qk[0], 0), (qk[1], 128), (qk[2], 256)):
                        inap = srct[0:128, 64 * w0:64 * w0 + 128]
                        # transpose as a regular bf16 matmul: in^T @ I
                        nc.tensor.matmul(pcps[:, oc:oc + 128], inap, idn[:, :],
                                         start=True, stop=True,
                                         skip_group_check=True)
                    nc.vector.tensor_copy(qkpc[:, 192 * p:192 * (p + 1)],
                                          pcps[:, 0:192])
                    nc.vector.tensor_copy(kzE[0:64, 192 * p:192 * (p + 1)],
                                          pcps[0:64, 192:384])
                    nc.scalar.copy(kzO[64:128, 192 * p:192 * (p + 1)],
                                   pcps[64:128, 192:384])
                    if p % 2 == 1:
                        yield
                # grams (transposed: G2[j,i]) + attention, quarters of 4 windows
                for quad in range(4):
                    gA = ps_gA.tile([128, 512], f32, tag="ga")
                    gB = ps_gB.tile([64, 512], f32, tag="gb")
                    for wq in range(4):
                        wl = 4 * quad + wq
                        p, wsub = wl // 2, wl % 2
                        kz = (kzE, kzO)[wsub]
                        qa = qkpc[0:128, 192 * p:192 * p + 128]
                        ka = kz[0:128, 192 * p:192 * p + 128]
                        nc.tensor.matmul(gA[:, 128 * wq:128 * wq + 128], ka, qa,
                                         start=(wq == 0), stop=(wq == 3),
                                         skip_group_check=True)
                        qb = qkpc[0:128, 192 * p + 128:192 * p + 192]
                        kb = kz[0:128, 192 * p + 128:192 * p + 192]
                        nc.tensor.matmul(gB[0:64, 64 * wq:64 * wq + 64], kb, qb,
                                         start=(wq == 0), stop=(wq == 3),
                                         skip_group_check=True)
                    eA = e_p.tile([128, 512], bf, tag="eA")
                    eB = e_p.tile([64, 256], bf, tag="eB")
                    nc.scalar.activation(eA[:], gA[:], AF.Exp, scale=tmpA[:])
                    nc.scalar.activation(eB[0:64, :], gB[0:64, 0:256], AF.Exp,
                                         scale=tmpB[:])
                    eAm = e_p.tile([128, 512], bf, tag="eAm")
                    eBm = e_p.tile([64, 256], bf, tag="eBm")
                    nc.vector.tensor_tensor(eAm[:], eA[:], mskA[:], op=AL.mult)
                    nc.vector.tensor_tensor(eBm[0:64, :], eB[0:64, :],
                                            mskB[:], op=AL.mult)
                    yield
                    oA = ps_oA.tile([128, 260], f32, tag="oa")
                    oB = ps_gB.tile([64, 512], f32, tag="gb")
                    for wq in range(4):
                        wl = 4 * quad + wq
                        wg = 16 * half + wl
                        nc.tensor.matmul(oA[0:128, 65 * wq:65 * wq + 65],
                                         eAm[:, 128 * wq:128 * wq + 128],
                                         v3[:, 65 * wg:65 * wg + 65],
                                         start=True, stop=True)
                        nc.tensor.matmul(oB[0:64, 65 * wq:65 * wq + 65],
                                         eBm[0:64, 64 * wq:64 * wq + 64],
                                         v4[0:64, 65 * wg:65 * wg + 65],
                                         start=True, stop=True)
                    oAv = oA[0:128, :].rearrange("p (w c) -> p w c", w=4)
                    oBv = oB[0:64, 0:260].rearrange("p (w c) -> p w c", w=4)
                    rzA = sm_p.tile([128, 4], f32, tag="rzA")
                    rzB = sm_p.tile([64, 4], f32, tag="rzB")
                    nc.vector.reciprocal(
                        rzA[:].rearrange("p (w c) -> p w c", c=1),
                        oAv[:, :, 64:65])
                    nc.vector.reciprocal(
                        rzB[0:64, :].rearrange("p (w c) -> p w c", c=1),
                        oBv[:, :, 64:65])
                    ob = 1024 * half + 256 * quad
                    nc.vector.scalar_tensor_tensor(
                        att_A[:, ob:ob + 256].rearrange("p (w c) -> p w c", w=4),
                        oAv[:, :, 0:64],
                        1.0,
                        rzA[:].unsqueeze(2).broadcast_to((128, 4, 64)),
                        AL.mult, AL.mult)
                    nc.vector.scalar_tensor_tensor(
                        att_B[0:64, ob:ob + 256].rearrange("p (w c) -> p w c",
                                                           w=4),
                        oBv[:, :, 0:64],
                        1.0,
                        rzB[0:64, :].unsqueeze(2).broadcast_to((64, 4, 64)),
                        AL.mult, AL.mult)
                    yield

            # ---- proj GEMM (restores flat pixel order) ----
            yA = y_p.tile([128, 2048], bf, tag="yA")
            yB = y_p.tile([64, 2048], bf, tag="yB")
            attAr = att_A[:, :].rearrange("p (w r c) -> p r w c", w=32, r=8)
            attBr = att_B[0:64, :].rearrange("p (w r c) -> p r w c", w=32, r=8)
            for nbp in range(2):
                ppAs = [ps_big.tile([128, 512], f32, tag="big",
                                    name=f"ppA{nbp}_{i}") for i in range(2)]
                ppBs = [ps_gB.tile([64, 512], f32, tag="gb",
                                   name=f"ppB{nbp}_{i}") for i in range(2)]
                for mo2, msz2, pps in ((0, 128, ppAs), (128, 64, ppBs)):
                    for kc in range(2):
                        lhsT = (pjA, pjB)[kc][:, mo2:mo2 + msz2]
                        for i, pp in enumerate(pps):
                            nb = 2 * nbp + i
                            rhs = (attAr, attBr)[kc][:, 2 * nb:2 * nb + 2, :, :]
                            nc.tensor.matmul(pp[0:msz2, :], lhsT, rhs,
                                             start=(kc == 0), stop=(kc == 1))
                for i in range(2):
                    nb = 2 * nbp + i
                    nc.vector.tensor_copy(yA[:, 512 * nb:512 * (nb + 1)],
                                          ppAs[i][:])
                    nc.scalar.copy(yB[0:64, 512 * nb:512 * (nb + 1)],
                                   ppBs[i][0:64, :])
                yield
            nc.sync.dma_start(y_e[0:128, 8 * s:8 * s + 8, :], yA[:])
            nc.sync.dma_start(y_e[128:192, 8 * s:8 * s + 8, :], yB[0:64, :])

        # ---- interleaved emission: att(s-1) units between A/B(s) units ----
        for s in range(NSTRIP + 1):
            g_att = gen_att(s - 1) if s >= 1 else None
            g_ab = gen_AB(s) if s < NSTRIP else None
            while g_att is not None or g_ab is not None:
                if g_att is not None:
                    try:
                        next(g_att)
                    except StopIteration:
                        g_att = None
                if g_ab is not None:
                    try:
                        next(g_ab)
                    except StopIteration:
                        g_ab = None

    nc.finalize()
    return nc


def _host_prep(inputs):
    x = np.ascontiguousarray(np.asarray(inputs["x"], dtype=np.float32))
    qkv_w = np.asarray(inputs["qkv_w"], dtype=np.float32)[:, :, 0, 0]
    dw_w = np.asarray(inputs["dw_w"], dtype=np.float32)[:, 0]
    proj_w = np.asarray(inputs["proj_w"], dtype=np.float32)[:, :, 0, 0]
    temp = np.asarray(inputs["temperature"], dtype=np.float32)[:, 0, 0]

    qT = np.ascontiguousarray(qkv_w.T)  # [192, 576]
    wqA = qT[0:128, :].astype(BF16)
    wqB = np.ascontiguousarray(qT[128:192, :]).astype(BF16)

    MCH = [(0, 128), (128, 128), (256, 128), (384, 128), (512, 64)]
    dgA = np.zeros((128, 36 * 128), np.float32)
    dgB = np.zeros((64, 9 * 64), np.float32)
    for m, (mo, msz) in enumerate(MCH):
        for t in range(9):
            d = np.diag(dw_w[mo:mo + msz, t // 3, t % 3])
            if m < 4:
                dgA[0:msz, (9 * m + t) * msz:(9 * m + t + 1) * msz] = d
            else:
                dgB[0:msz, t * msz:(t + 1) * msz] = d
    dgA = dgA.astype(BF16)
    dgB = dgB.astype(BF16)

    projT = np.ascontiguousarray(proj_w.T)
    pjA = np.ascontiguousarray(projT[0:128, :]).astype(BF16)
    pjB = np.ascontiguousarray(projT[128:192, :]).astype(BF16)
    tmpA = np.repeat(temp[0:4], 32).reshape(128, 1).astype(np.float32)
    tmpB = np.repeat(temp[4:6], 32).reshape(64, 1).astype(np.float32)
    idn = np.eye(128, dtype=np.float32).astype(BF16)
    mskA = np.zeros((128, 512), np.float32)
    for g in range(4):
        for rep in range(4):
            mskA[32 * g:32 * g + 32,
                 128 * rep + 32 * g:128 * rep + 32 * g + 32] = 1.0
    mskB = np.zeros((64, 256), np.float32)
    for g in range(2):
        for rep in range(4):
            mskB[32 * g:32 * g + 32,
                 64 * rep + 32 * g:64 * rep + 32 * g + 32] = 1.0
    mskA = mskA.astype(BF16)
    mskB = mskB.astype(BF16)

    xpad = np.pad(x, ((0, 0), (0, 0), (1, 1), (1, 1)))
    in_maps = []
    for core in range(NCORES):
        b, halfc = core // 2, core % 2
        r0 = 128 * halfc
        xp = np.ascontiguousarray(xpad[b, :, r0:r0 + 130, :]).astype(BF16)
        in_maps.append(dict(xp=xp, wqA=wqA, wqB=wqB, dgA=dgA, dgB=dgB,
                            pjA=pjA, pjB=pjB, tmpA=tmpA, tmpB=tmpB, idn=idn,
                            mskA=mskA, mskB=mskB))
    return in_maps


def kernel(**inputs):
    global LAST_RESULT
    from concourse.bass_utils import run_bass_kernel_spmd

    in_maps = _host_prep(inputs)

    if "nc" not in _CACHE:
        _CACHE["nc"] = _build_nc()
    nc = _CACHE["nc"]

    res = run_bass_kernel_spmd(nc, in_maps, list(range(NCORES)))
    LAST_RESULT = res

    out = np.zeros((B, DIM, H, W), np.float32)
    for core in range(NCORES):
        b, halfc = core // 2, core % 2
        r0 = 128 * halfc
        out[b, :, r0:r0 + 128, :] = res.results[core]["y"].astype(np.float32)
    return out


# revision 23
# speedup vs baseline: 1.1983x; 1.0000x over previous
"""Trainium2 Bass kernel for nn_Channel_Attention (XCA-style windowed channel attention).

v3 = v2 + software pipelining: the attention phase of strip s-1 is emitted
interleaved with the qkv/depthwise GEMM stages of strip s (disjoint PSUM
banks, double-buffered qk/v tiles), so the PE never idles long enough for
HAM to re-throttle and DVE/ACT chains hide under PE work.

Kernel math (per core, data-parallel over batch x H-half):
  - stage A: qkv1 = conv1x1(x) on a 10-row halo strip (bf16 GEMM, K=192)
  - stage B: qkv_dw = sum_t diag(dw[:,t]) @ shift_t(qkv1)  (9 accumulating bf16
    matmuls per chunk, K = chunk size <= 128)
  - per-(chan,window) l2 norms: ACT Square + DVE window reduce + Ln + Exp(-0.5 ln)
  - q,k evacuated f32 with fused 1/norm scaling (DVE stt, broadcast AP)
  - per-window pc layout via PE transpose (f32: bf16 PSUM is broken on trn2 HW)
  - transposed gram G2[j,i] with k zero-padded to K=128 (bf16 accum groups with
    mixed tile positions crash trn2; all matmuls full-K at position (0,0))
  - exp feeds out-matmul as lhsT directly; ones column in v gives Z in-matmul
  - 1/Z fused into evacuation; final proj GEMM restores flat pixel order
"""
import sys

sys.path.insert(0, "/opt/trn_rl_repo")

import numpy as np
import ml_dtypes

BF16 = ml_dtypes.bfloat16

DIM, HEADS, PS = 192, 6, 8
B, H, W = 4, 256, 256
C3 = 3 * DIM  # 576
NCORES = 8
ROWS = 128
NSTRIP = 16

_CACHE = {}
LAST_RESULT = None


def _build_nc():
    import contextlib

    import concourse.mybir as mybir
    import concourse.tile as tile
    from concourse import bacc

    f32 = mybir.dt.float32
    bf = mybir.dt.bfloat16
    AL = mybir.AluOpType
    AF = mybir.ActivationFunctionType
    AX = mybir.AxisListType

    nc = bacc.Bacc("TRN2", target_bir_lowering=False, debug=False, num_devices=NCORES)

    xp_e = nc.declare_dram_parameter("xp", [DIM, 130, 258], bf, isOutput=False)
    wqA_e = nc.declare_dram_parameter("wqA", [128, C3], bf, isOutput=False)
    wqB_e = nc.declare_dram_parameter("wqB", [64, C3], bf, isOutput=False)
    dgA_e = nc.declare_dram_parameter("dgA", [128, 36 * 128], bf, isOutput=False)
    dgB_e = nc.declare_dram_parameter("dgB", [64, 9 * 64], bf, isOutput=False)
    pjA_e = nc.declare_dram_parameter("pjA", [128, 192], bf, isOutput=False)
    pjB_e = nc.declare_dram_parameter("pjB", [64, 192], bf, isOutput=False)
    tmpA_e = nc.declare_dram_parameter("tmpA", [128, 1], f32, isOutput=False)
    tmpB_e = nc.declare_dram_parameter("tmpB", [64, 1], f32, isOutput=False)
    idn_e = nc.declare_dram_parameter("idn", [128, 128], bf, isOutput=False)
    mskA_e = nc.declare_dram_parameter("mskA", [128, 512], bf, isOutput=False)
    mskB_e = nc.declare_dram_parameter("mskB", [64, 256], bf, isOutput=False)
    y_e = nc.declare_dram_parameter("y", [DIM, ROWS, 256], bf, isOutput=True)

    with tile.TileContext(nc) as tc, contextlib.ExitStack() as ctx:
        const = ctx.enter_context(tc.tile_pool(name="const", bufs=1))
        xs_p = ctx.enter_context(tc.tile_pool(name="xs", bufs=2))
        q1_p = ctx.enter_context(tc.tile_pool(name="q1", bufs=1))
        qk_p = ctx.enter_context(tc.tile_pool(name="qk", bufs=2))
        pc_p = ctx.enter_context(tc.tile_pool(name="pc", bufs=1))
        sm_p = ctx.enter_context(tc.tile_pool(name="small", bufs=2))
        stg_p = ctx.enter_context(tc.tile_pool(name="stg", bufs=2))
        e_p = ctx.enter_context(tc.tile_pool(name="eb", bufs=2))
        att_p = ctx.enter_context(tc.tile_pool(name="att", bufs=1))
        y_p = ctx.enter_context(tc.tile_pool(name="yb", bufs=1))
        ps_big = ctx.enter_context(tc.tile_pool(name="psbig", bufs=4, space="PSUM"))
        ps_pct = ctx.enter_context(tc.tile_pool(name="pspct", bufs=2, space="PSUM"))
        ps_gA = ctx.enter_context(tc.tile_pool(name="psga", bufs=1, space="PSUM"))
        ps_gB = ctx.enter_context(tc.tile_pool(name="psgb", bufs=1, space="PSUM"))

        # ---- constants ----
        wqA = const.tile([128, C3], bf)
        nc.sync.dma_start(wqA[:], wqA_e[:, :])
        wqB = const.tile([64, C3], bf)
        nc.sync.dma_start(wqB[:], wqB_e[:, :])
        dgA = const.tile([128, 36 * 128], bf)
        nc.sync.dma_start(dgA[:], dgA_e[:, :])
        dgB = const.tile([64, 9 * 64], bf)
        nc.sync.dma_start(dgB[:], dgB_e[:, :])
        pjA = const.tile([128, 192], bf)
        nc.sync.dma_start(pjA[:], pjA_e[:, :])
        pjB = const.tile([64, 192], bf)
        nc.sync.dma_start(pjB[:], pjB_e[:, :])
        tmpA = const.tile([128, 1], f32)
        nc.sync.dma_start(tmpA[:], tmpA_e[:, :])
        tmpB = const.tile([64, 1], f32)
        nc.sync.dma_start(tmpB[:], tmpB_e[:, :])
        idn = const.tile([128, 128], bf)
        nc.sync.dma_start(idn[:], idn_e[:, :])
        mskA = const.tile([128, 512], bf)
        nc.sync.dma_start(mskA[:], mskA_e[:, :])
        mskB = const.tile([64, 256], bf)
        nc.sync.dma_start(mskB[:], mskB_e[:, :])

        MCH = [(0, 128), (128, 128), (256, 128), (384, 128), (512, 64)]

        # k in pc layout, zero-padded to full 128 partitions per window parity
        kzE = pc_p.tile([128, 8 * 192], bf, name="kzE")
        kzO = pc_p.tile([128, 8 * 192], bf, name="kzO")
        nc.vector.memset(kzE[64:128, :], 0.0)
        nc.vector.memset(kzO[0:64, :], 0.0)

        # qkv1 (pre-depthwise) chunk tiles: [chans, 10 rows x 258 cols]
        q1 = [q1_p.tile([128, 2580], bf, name=f"q1c{m}") for m in range(4)]
        q1.append(q1_p.tile([64, 2580], bf, name="q1c4"))
        q1v = [t[0:msz, :].rearrange("p (r c) -> p r c", r=10)
               for t, (mo, msz) in zip(q1, MCH)]

        state = {}  # per-strip tiles handed from gen_AB(s) to gen_att(s)

        def gen_AB(s):
            # ---- load x strip (10 rows x 258 cols, padded) ----
            xsA = xs_p.tile([128, 10 * 258], bf, tag="xsA")
            nc.sync.dma_start(xsA[:], xp_e[0:128, 8 * s:8 * s + 10, :])
            xsB = xs_p.tile([64, 10 * 258], bf, tag="xsB")
            nc.sync.dma_start(xsB[:], xp_e[128:192, 8 * s:8 * s + 10, :])

            qk = [qk_p.tile([128, 2048], bf, name=f"qk{m}", tag=f"qk{m}")
                  for m in range(3)]
            # v tiles: 65-col window stride, col 64 of each window = 1.0 so
            # out-matmuls against [v | 1] produce Z in the extra column.
            v3 = qk_p.tile([128, 32 * 65], bf, tag="v3")
            v4 = qk_p.tile([64, 32 * 65], bf, tag="v4")
            v3w = v3[:].rearrange("p (w c) -> p w c", w=32)
            v4w = v4[0:64, :].rearrange("p (w c) -> p w c", w=32)
            nc.vector.memset(v3w[:, :, 64:65], 1.0)
            nc.vector.memset(v4w[:, :, 64:65], 1.0)
            state[s] = (qk, v3, v4)

            # ---- stage A: qkv1 = conv1x1(x) ----
            # Strip 0 computes the full 10-row halo strip; later strips reuse
            # the previous strip's last 2 rows (q1 rows 8,9 == this strip's
            # rows 0,1) via an SBUF copy and compute only the 8 new rows.
            if s == 0:
                blocks = [(i * 430, 430) for i in range(6)]
            else:
                for m, (mo, msz) in enumerate(MCH):
                    nc.vector.tensor_copy(q1[m][0:msz, 0:516],
                                          q1[m][0:msz, 2064:2580])
                blocks = [(516 + i * 344, 344) for i in range(6)]
            for m, (mo, msz) in enumerate(MCH):
                for ng in range(3):
                    psq = [ps_big.tile([128, 512], f32, tag="big",
                                       name=f"psq{m}_{ng}_{i}")
                           for i in range(2)]
                    for kc in range(2):
                        wq = (wqA, wqB)[kc]
                        xf = (xsA, xsB)[kc]
                        for nb, ps in enumerate(psq):
                            c0, wdt = blocks[2 * ng + nb]
                            nc.tensor.matmul(
                                ps[0:msz, 0:wdt], wq[:, mo:mo + msz],
                                xf[:, c0:c0 + wdt],
                                start=(kc == 0), stop=(kc == 1))
                    for nb, ps in enumerate(psq):
                        c0, wdt = blocks[2 * ng + nb]
                        if nb == 0 and ng != 1:
                            nc.vector.tensor_copy(q1[m][0:msz, c0:c0 + wdt],
                                                  ps[0:msz, 0:wdt])
                        else:
                            nc.scalar.copy(q1[m][0:msz, c0:c0 + wdt],
                                           ps[0:msz, 0:wdt])
                    yield

            # ---- stage B: depthwise via diag matmuls + norms + evac ----
            # N-block j = 8 rows x cols [64j, 64j+64) = windows [8j, 8j+8)
            for m, (mo, msz) in enumerate(MCH):
                dg, dbase = (dgA, 9 * m) if m < 4 else (dgB, 0)
                psl = []
                for j in range(4):
                    ps = ps_big.tile([128, 512], f32, tag="big",
                                     name=f"psb{m}_{j}")
                    psl.append((j, ps))
                for t in range(9):
                    dy, dx = t // 3, t % 3
                    lhsT = dg[:, (dbase + t) * msz:(dbase + t + 1) * msz]
                    for j, ps in psl:
                        rhs = q1v[m][:, dy:dy + 8,
                                     64 * j + dx:64 * j + dx + 64
                                     ].rearrange("p r (w c) -> p w r c", w=8)
                        nc.tensor.matmul(
                            ps[0:msz, :], lhsT, rhs,
                            start=(t == 0), stop=(t == 8))
                if m < 3:
                    # fast staging copy frees the PSUM banks immediately;
                    # norms + scaled evac then read SBUF
                    stg = stg_p.tile([128, 2048], bf, tag="stg")
                    for j, ps in psl:
                        if j < 2:
                            nc.vector.tensor_copy(
                                stg[0:msz, 512 * j:512 * (j + 1)], ps[0:msz, :])
                        else:
                            nc.scalar.copy(
                                stg[0:msz, 512 * j:512 * (j + 1)], ps[0:msz, :])
                    yield
                    n2 = sm_p.tile([128, 32], f32, tag="n2")
                    for j, ps in psl:
                        sq = sm_p.tile([128, 512], f32, tag="sq")
                        nc.scalar.activation(sq[0:msz, :],
                                             stg[0:msz, 512 * j:512 * (j + 1)],
                                             AF.Square)
                        sqv = sq[0:msz, :].rearrange("p (w c) -> p w c", w=8)
                        nc.vector.tensor_reduce(
                            n2[0:msz, 8 * j:8 * j + 8], sqv,
                            axis=AX.X, op=AL.add)
                    yield
                    lnn = sm_p.tile([128, 32], f32, tag="nrm")
                    nc.scalar.activation(lnn[0:msz, :], n2[0:msz, :],
                                         AF.Ln)
                    rj = sm_p.tile([128, 32], f32, tag="rj")
                    nc.scalar.activation(rj[0:msz, :], lnn[0:msz, :],
                                         AF.Exp, scale=-0.5)
                    for j, ps in psl:
                        # evac with 1/norm scaling (win-grouped layout)
                        dst = qk[m][:, 512 * j:512 * (j + 1)].rearrange(
                            "p (w c) -> p w c", w=8)
                        nc.vector.scalar_tensor_tensor(
                            dst,
                            stg[0:msz, 512 * j:512 * (j + 1)].rearrange(
                                "p (w c) -> p w c", w=8),
                            1.0,
                            rj[0:msz, 8 * j:8 * j + 8].unsqueeze(2)
                            .broadcast_to((msz, 8, 64)),
                            AL.mult, AL.mult)
                    yield
                else:
                    for j, ps in psl:
                        vw = (v3w, v4w)[m - 3]
                        nc.scalar.copy(
                            vw[0:msz, 8 * j:8 * j + 8, 0:64],
                            ps[0:msz, :].rearrange("p (w c) -> p w c", w=8))
                    yield

        def gen_att(s):
            qk, v3, v4 = state.pop(s)
            # ---- pc transposes + attention, halves of 16 windows ----
            for half in range(2):
                if half == 0:
                    att_A = att_p.tile([128, 2048], bf, tag="attA")
                    att_B = att_p.tile([64, 2048], bf, tag="attB")
                qkpc = pc_p.tile([128, 8 * 192], bf, tag="qkpc")
                for p in range(8):
                    w0 = 16 * half + 2 * p
                    pcps = ps_pct.tile([128, 384], f32, tag="pct")
                    for (srct, oc) in ((qk[0], 0), (qk[1], 128), (qk[2], 256)):
                        inap = srct[0:128, 64 * w0:64 * w0 + 128]
                        # transpose as a regular bf16 matmul: in^T @ I
                        nc.tensor.matmul(pcps[:, oc:oc + 128], inap, idn[:, :],
                                         start=True, stop=True,
                                         skip_group_check=True)
                    nc.vector.tensor_copy(qkpc[:, 192 * p:192 * (p + 1)],
                                          pcps[:, 0:192])
                    nc.vector.tensor_copy(kzE[0:64, 192 * p:192 * (p + 1)],
                                          pcps[0:64, 192:384])
                    nc.scalar.copy(kzO[64:128, 192 * p:192 * (p + 1)],
                                   pcps[64:128, 192:384])
                    if p % 2 == 1:
                        yield
                # grams (transposed: G2[j,i]) + attention, quarters of 4 windows
                for quad in range(4):
                    gA = ps_gA.tile([128, 512], f32, tag="ga")
                    gB = ps_gB.tile([64, 512], f32, tag="gb")
                    for wq in range(4):
                        wl = 4 * quad + wq
                        p, wsub = wl // 2, wl % 2
                        kz = (kzE, kzO)[wsub]
                        qa = qkpc[0:128, 192 * p:192 * p + 128]
                        ka = kz[0:128, 192 * p:192 * p + 128]
                        nc.tensor.matmul(gA[:, 128 * wq:128 * wq + 128], ka, qa,
                                         start=(wq == 0), stop=(wq == 3),
                                         skip_group_check=True)
                        qb = qkpc[0:128, 192 * p + 128:192 * p + 192]
                        kb = kz[0:128, 192 * p + 128:192 * p + 192]
                        nc.tensor.matmul(gB[0:64, 64 * wq:64 * wq + 64], kb, qb,
                                         start=(wq == 0), stop=(wq == 3),
                                         skip_group_check=True)
                    eA = e_p.tile([128, 512], bf, tag="eA")
                    eB = e_p.tile([64, 256], bf, tag="eB")
                    nc.scalar.activation(eA[:], gA[:], AF.Exp, scale=tmpA[:])
                    nc.scalar.activation(eB[0:64, :], gB[0:64, 0:256], AF.Exp,
                                         scale=tmpB[:])
                    eAm = e_p.tile([128, 512], bf, tag="eAm")
                    eBm = e_p.tile([64, 256], bf, tag="eBm")
                    nc.vector.tensor_tensor(eAm[:], eA[:], mskA[:], op=AL.mult)
                    nc.vector.tensor_tensor(eBm[0:64, :], eB[0:64, :],
                                            mskB[:], op=AL.mult)
                    yield
                    oA = ps_gA.tile([128, 512], f32, tag="ga",
                                    name=f"oa{quad}")
                    oB = ps_gB.tile([64, 512], f32, tag="gb")
                    for wq in range(4):
                        wl = 4 * quad + wq
                        wg = 16 * half + wl
                        nc.tensor.matmul(oA[0:128, 65 * wq:65 * wq + 65],
                                         eAm[:, 128 * wq:128 * wq + 128],
                                         v3[:, 65 * wg:65 * wg + 65],
                                         start=True, stop=True)
                        nc.tensor.matmul(oB[0:64, 65 * wq:65 * wq + 65],
                                         eBm[0:64, 64 * wq:64 * wq + 64],
                                         v4[0:64, 65 * wg:65 * wg + 65],
                                         start=True, stop=True)
                    oAv = oA[0:128, 0:260].rearrange("p (w c) -> p w c", w=4)
                    oBv = oB[0:64, 0:260].rearrange("p (w c) -> p w c", w=4)
                    rzA = sm_p.tile([128, 4], f32, tag="rzA")
                    rzB = sm_p.tile([64, 4], f32, tag="rzB")
                    nc.vector.reciprocal(
                        rzA[:].rearrange("p (w c) -> p w c", c=1),
                        oAv[:, :, 64:65])
                    nc.vector.reciprocal(
                        rzB[0:64, :].rearrange("p (w c) -> p w c", c=1),
                        oBv[:, :, 64:65])
                    ob = 1024 * half + 256 * quad
                    nc.vector.scalar_tensor_tensor(
                        att_A[:, ob:ob + 256].rearrange("p (w c) -> p w c", w=4),
                        oAv[:, :, 0:64],
                        1.0,
                        rzA[:].unsqueeze(2).broadcast_to((128, 4, 64)),
                        AL.mult, AL.mult)
                    nc.vector.scalar_tensor_tensor(
                        att_B[0:64, ob:ob + 256].rearrange("p (w c) -> p w c",
                                                           w=4),
                        oBv[:, :, 0:64],
                        1.0,
                        rzB[0:64, :].unsqueeze(2).broadcast_to((64, 4, 64)),
                        AL.mult, AL.mult)
                    yield

            # ---- proj GEMM (restores flat pixel order) ----
            yA = y_p.tile([128, 2048], bf, tag="yA")
            yB = y_p.tile([64, 2048], bf, tag="yB")
            attAr = att_A[:, :].rearrange("p (w r c) -> p r w c", w=32, r=8)
            attBr = att_B[0:64, :].rearrange("p (w r c) -> p r w c", w=32, r=8)
            for nbp in range(2):
                ppAs = [ps_big.tile([128, 512], f32, tag="big",
                                    name=f"ppA{nbp}_{i}") for i in range(2)]
                ppBs = [ps_gB.tile([64, 512], f32, tag="gb",
                                   name=f"ppB{nbp}_{i}") for i in range(2)]
                for mo2, msz2, pps in ((0, 128, ppAs), (128, 64, ppBs)):
                    for kc in range(2):
                        lhsT = (pjA, pjB)[kc][:, mo2:mo2 + msz2]
                        for i, pp in enumerate(pps):
                            nb = 2 * nbp + i
                            rhs = (attAr, attBr)[kc][:, 2 * nb:2 * nb + 2, :, :]
                            nc.tensor.matmul(pp[0:msz2, :], lhsT, rhs,
                                             start=(kc == 0), stop=(kc == 1))
                for i in range(2):
                    nb = 2 * nbp + i
                    nc.vector.tensor_copy(yA[:, 512 * nb:512 * (nb + 1)],
                                          ppAs[i][:])
                    nc.scalar.copy(yB[0:64, 512 * nb:512 * (nb + 1)],
                                   ppBs[i][0:64, :])
                yield
            nc.sync.dma_start(y_e[0:128, 8 * s:8 * s + 8, :], yA[:])
            nc.sync.dma_start(y_e[128:192, 8 * s:8 * s + 8, :], yB[0:64, :])

        # ---- interleaved emission: att(s-1) units between A/B(s) units ----
        for s in range(NSTRIP + 1):
            g_att = gen_att(s - 1) if s >= 1 else None
            g_ab = gen_AB(s) if s < NSTRIP else None
            while g_att is not None or g_ab is not None:
                if g_att is not None:
                    try:
                        next(g_att)
                    except StopIteration:
                        g_att = None
                if g_ab is not None:
                    try:
                        next(g_ab)
                    except StopIteration:
                        g_ab = None

    nc.finalize()
    return nc


def _host_prep(inputs):
    x = np.ascontiguousarray(np.asarray(inputs["x"], dtype=np.float32))
    qkv_w = np.asarray(inputs["qkv_w"], dtype=np.float32)[:, :, 0, 0]
    dw_w = np.asarray(inputs["dw_w"], dtype=np.float32)[:, 0]
    proj_w = np.asarray(inputs["proj_w"], dtype=np.float32)[:, :, 0, 0]
    temp = np.asarray(inputs["temperature"], dtype=np.float32)[:, 0, 0]

    qT = np.ascontiguousarray(qkv_w.T)  # [192, 576]
    wqA = qT[0:128, :].astype(BF16)
    wqB = np.ascontiguousarray(qT[128:192, :]).astype(BF16)

    MCH = [(0, 128), (128, 128), (256, 128), (384, 128), (512, 64)]
    dgA = np.zeros((128, 36 * 128), np.float32)
    dgB = np.zeros((64, 9 * 64), np.float32)
    for m, (mo, msz) in enumerate(MCH):
        for t in range(9):
            d = np.diag(dw_w[mo:mo + msz, t // 3, t % 3])
            if m < 4:
                dgA[0:msz, (9 * m + t) * msz:(9 * m + t + 1) * msz] = d
            else:
                dgB[0:msz, t * msz:(t + 1) * msz] = d
    dgA = dgA.astype(BF16)
    dgB = dgB.astype(BF16)

    projT = np.ascontiguousarray(proj_w.T)
    pjA = np.ascontiguousarray(projT[0:128, :]).astype(BF16)
    pjB = np.ascontiguousarray(projT[128:192, :]).astype(BF16)
    tmpA = np.repeat(temp[0:4], 32).reshape(128, 1).astype(np.float32)
    tmpB = np.repeat(temp[4:6], 32).reshape(64, 1).astype(np.float32)
    idn = np.eye(128, dtype=np.float32).astype(BF16)
    mskA = np.zeros((128, 512), np.float32)
    for g in range(4):
        for rep in range(4):
            mskA[32 * g:32 * g + 32,
                 128 * rep + 32 * g:128 * rep + 32 * g + 32] = 1.0
    mskB = np.zeros((64, 256), np.float32)
    for g in range(2):
        for rep in range(4):
            mskB[32 * g:32 * g + 32,
                 64 * rep + 32 * g:64 * rep + 32 * g + 32] = 1.0
    mskA = mskA.astype(BF16)
    mskB = mskB.astype(BF16)

    xpad = np.pad(x, ((0, 0), (0, 0), (1, 1), (1, 1)))
    in_maps = []
    for core in range(NCORES):
        b, halfc = core // 2, core % 2
        r0 = 128 * halfc
        xp = np.ascontiguousarray(xpad[b, :, r0:r0 + 130, :]).astype(BF16)
        in_maps.append(dict(xp=xp, wqA=wqA, wqB=wqB, dgA=dgA, dgB=dgB,
                            pjA=pjA, pjB=pjB, tmpA=tmpA, tmpB=tmpB, idn=idn,
                            mskA=mskA, mskB=mskB))
    return in_maps


def kernel(**inputs):
    global LAST_RESULT
    from concourse.bass_utils import run_bass_kernel_spmd

    in_maps = _host_prep(inputs)

    if "nc" not in _CACHE:
        _CACHE["nc"] = _build_nc()
    nc = _CACHE["nc"]

    res = run_bass_kernel_spmd(nc, in_maps, list(range(NCORES)))
    LAST_RESULT = res

    out = np.zeros((B, DIM, H, W), np.float32)
    for core in range(NCORES):
        b, halfc = core // 2, core % 2
        r0 = 128 * halfc
        out[b, :, r0:r0 + 128, :] = res.results[core]["y"].astype(np.float32)
    return out
